# revision 1
# baseline (speedup 1.0000x reference)
"""GCN shallow regression kernel for 8 TRN2 NeuronCores.

Strategy (graph partitioned by destination node range, 12500 nodes/core):
  reference: out = sigmoid(relu(A_norm @ (x @ W.T) + b) @ lin_w.T + lin_b)
  We use A_norm @ (x @ W.T) == (A_norm @ x) @ W.T and aggregate raw x rows.

  Host: add self loops, compute norm = dinv[src]*dinv[dst], sort edges by
  dst, split by dst range into 8 cores, group edges into 128-edge chunks
  where each chunk targets one 128-node dst window.  Within a window,
  edges are grouped by source range (32768 nodes per range) so the
  dma_gather int16 indices fit; each (window, range) cell is padded to
  whole chunks.  All cores share one static schedule (cell sizes = max
  over cores).

  Device, per window w (PSUM accumulation over its chunks):
    G[e, ci]   = x_bf16[src[e], ci]                 (dma_gather per range cell)
    OH[e, d]   = (iota[d] == dstoff[e]) * norm[e]   (one DVE op per chunk)
    aggT[ci,d] += G.T @ OH                          (PE, bf16 -> f32 PSUM)
    h[co, d]   = W^T.T @ aggT                       (PE, f32)
    r[co, d]   = relu(h + conv_bias[co])            (ACT)
    o[d]       = r.T @ lin_w                        (PE, f32)
    out[d]     = sigmoid(o + lin_b)                 (ACT)
"""

import sys

if "/opt/trn_rl_repo" not in sys.path:
    sys.path.insert(0, "/opt/trn_rl_repo")

import numpy as np
import ml_dtypes

from concourse import bacc, bass, mybir
from concourse.bass_utils import run_bass_kernel_spmd
from concourse.tile import TileContext

P = 128
NCORES = 8
RANGE = 32768          # dma_gather int16 index range per source slice
NQ = 4                 # SWDGE queues (Q7 core pairs) to spread gathers over
F32 = mybir.dt.float32
BF16 = mybir.dt.bfloat16
I16 = mybir.dt.int16


def preprocess(x, edge_index, W, conv_bias, lin_w, lin_b, ncores=NCORES):
    """Host-side sharding. Returns (cpwr, in_maps, npc, nwin)."""
    x = np.asarray(x)
    edge_index = np.asarray(edge_index)
    N = x.shape[0]
    npc = -(-N // ncores)          # nodes per core
    nwin = -(-npc // P)            # dst windows per core
    nrange = -(-N // RANGE)        # source ranges

    loop = np.arange(N, dtype=np.int64)
    src = np.concatenate([edge_index[0].astype(np.int64), loop])
    dst = np.concatenate([edge_index[1].astype(np.int64), loop])
    deg = np.bincount(dst, minlength=N).astype(np.float64)
    dinv = 1.0 / np.sqrt(deg)
    norm = (dinv[src] * dinv[dst]).astype(np.float32)

    rng_s = src // RANGE
    # order edges by (core, window, range); stable so positions are easy
    core_k = dst // npc
    win_k = (dst % npc) // P
    key = (core_k * nwin + win_k) * nrange + rng_s
    order = np.argsort(key, kind="stable")
    src_s, dst_s, norm_s, key_s = src[order], dst[order], norm[order], key[order]
    off_s = (dst_s % npc) % P
    rng_ss = rng_s[order]

    ncells = ncores * nwin * nrange
    cnt = np.bincount(key_s, minlength=ncells).reshape(ncores, nwin, nrange)
    cpwr = (-(-cnt // P)).max(axis=0)          # [nwin, nrange] chunks per cell
    cpw = cpwr.sum(axis=1)                     # [nwin] chunks per window
    TC = int(cpw.sum())
    # slot base (in chunks) for each (window, range) cell
    cell_base = np.concatenate([[0], np.cumsum(cpwr.reshape(-1))[:-1]]).reshape(
        nwin, nrange
    )

    # position of each edge in its core's slot array
    seg_start = np.searchsorted(key_s, np.arange(ncells, dtype=np.int64))
    idx_in_cell = np.arange(len(dst_s), dtype=np.int64) - seg_start[key_s]
    wr = key_s % (nwin * nrange)
    pos = cell_base.reshape(-1)[wr] * P + idx_in_cell
    core_s = key_s // (nwin * nrange)

    # int16 wrapped index layout: per cell, ordinal k -> [k%16, 16 reps][k//16]
    # Flattened free dim: chunk slot s covers int16 columns [s*8, (s+1)*8).
    xb = np.ascontiguousarray(x.astype(ml_dtypes.bfloat16))
    wt = np.ascontiguousarray(np.asarray(W, np.float32).T)          # [ci, co]
    bias_col = np.asarray(conv_bias, np.float32).reshape(P, 1)
    linw_col = np.asarray(lin_w, np.float32).reshape(P, 1)
    linb_col = np.full((P, 1), np.float32(np.asarray(lin_b).reshape(-1)[0]))
    iota = np.ascontiguousarray(
        np.broadcast_to(np.arange(P, dtype=np.float32), (P, P)).astype(
            ml_dtypes.bfloat16
        )
    )

    in_maps = []
    for c in range(ncores):
        m = core_s == c
        posm = pos[m]
        srci = np.zeros(TC * P, dtype=np.int16)
        dstoff = np.zeros(TC * P, dtype=np.float32)
        normq = np.zeros(TC * P, dtype=np.float32)
        srci[posm] = (src_s[m] - rng_ss[m] * RANGE).astype(np.int16)
        dstoff[posm] = off_s[m]
        normq[posm] = norm_s[m]
        # slot arrays -> SBUF layouts
        # dstoff/normq: [P(lane), TC(chunk)]
        dstoff = np.ascontiguousarray(dstoff.reshape(TC, P).T)
        normq = np.ascontiguousarray(normq.reshape(TC, P).T.astype(ml_dtypes.bfloat16))
        # srci wrapped: ordinal k within the whole array; since cells are
        # chunk-aligned and the wrap stride (16) divides P, wrapping the whole
        # array at once equals per-cell wrapping.
        w16 = srci.reshape(TC * 8, 16).T               # [16, TC*8]
        srci16 = np.ascontiguousarray(np.tile(w16, (8, 1)))  # [128, TC*8]
        in_maps.append(
            {
                "xb": xb,
                "srcix": srci16,
                "dstoff": dstoff,
                "normq": normq,
                "wt": wt,
                "bias": bias_col,
                "linw": linw_col,
                "linb": linb_col,
                "iota": iota,
            }
        )
    return cpwr, in_maps, npc, nwin


def build(cpwr, N):
    """Build + compile the per-core Bass kernel (same NEFF for all cores)."""
    nwin, nrange = cpwr.shape
    cpw = cpwr.sum(axis=1)
    TC = int(cpw.sum())
    nc = bacc.Bacc(
        None, target_bir_lowering=False, debug=False, num_swdge_queues=NQ
    )

    xb = nc.dram_tensor("xb", [N, P], BF16, kind="ExternalInput")
    srcix = nc.dram_tensor("srcix", [P, TC * 8], I16, kind="ExternalInput")
    dstoff = nc.dram_tensor("dstoff", [P, TC], F32, kind="ExternalInput")
    normq = nc.dram_tensor("normq", [P, TC], BF16, kind="ExternalInput")
    wt = nc.dram_tensor("wt", [P, P], F32, kind="ExternalInput")
    bias = nc.dram_tensor("bias", [P, 1], F32, kind="ExternalInput")
    linw = nc.dram_tensor("linw", [P, 1], F32, kind="ExternalInput")
    linb = nc.dram_tensor("linb", [P, 1], F32, kind="ExternalInput")
    iota = nc.dram_tensor("iota", [P, P], BF16, kind="ExternalInput")
    out = nc.dram_tensor("out", [nwin * P, 1], F32, kind="ExternalOutput")

    gq = 0  # round-robin gather queue
    with TileContext(nc) as tc:
        with (
            tc.tile_pool(name="const", bufs=1) as cpool,
            tc.tile_pool(name="meta", bufs=3) as mpool,
            tc.tile_pool(name="g", bufs=3) as gpool,
            tc.tile_pool(name="oh", bufs=6) as ohpool,
            tc.tile_pool(name="ep", bufs=2) as eppool,
            tc.tile_pool(name="psA", bufs=2, space="PSUM") as psA,
            tc.tile_pool(name="psB", bufs=2, space="PSUM") as psB,
            tc.tile_pool(name="psC", bufs=2, space="PSUM") as psC,
        ):
            wt_sb = cpool.tile([P, P], F32, tag="wt")
            nc.sync.dma_start(out=wt_sb[:], in_=wt[:])
            bias_sb = cpool.tile([P, 1], F32, tag="bias")
            nc.sync.dma_start(out=bias_sb[:], in_=bias[:])
            linw_sb = cpool.tile([P, 1], F32, tag="linw")
            nc.sync.dma_start(out=linw_sb[:], in_=linw[:])
            linb_sb = cpool.tile([P, 1], F32, tag="linb")
            nc.sync.dma_start(out=linb_sb[:], in_=linb[:])
            iota_sb = cpool.tile([P, P], BF16, tag="iota")
            nc.sync.dma_start(out=iota_sb[:], in_=iota[:])

            cbase = 0
            for w in range(nwin):
                cw = int(cpw[w])
                ix_sb = mpool.tile([P, cw * 8], I16, tag="ix")
                do_sb = mpool.tile([P, cw], F32, tag="do")
                nq_sb = mpool.tile([P, cw], BF16, tag="nq")
                nc.sync.dma_start(
                    out=ix_sb[:], in_=srcix[:, cbase * 8 : (cbase + cw) * 8]
                )
                nc.sync.dma_start(out=do_sb[:], in_=dstoff[:, cbase : cbase + cw])
                nc.sync.dma_start(out=nq_sb[:], in_=normq[:, cbase : cbase + cw])

                g_sb = gpool.tile([P, cw * P], BF16, tag="g")
                off = 0
                for r in range(nrange):
                    cwr = int(cpwr[w, r])
                    if cwr == 0:
                        continue
                    rbase = r * RANGE
                    rlen = min(RANGE, N - rbase)
                    nc.gpsimd.dma_gather(
                        g_sb[:, off * P : (off + cwr) * P].rearrange(
                            "p (c e) -> p c e", e=P
                        ),
                        xb[rbase : rbase + rlen, :],
                        ix_sb[:, off * 8 : (off + cwr) * 8],
                        cwr * P,
                        cwr * P,
                        P,
                        single_packet=False,
                        queue_num=gq % NQ,
                    )
                    gq += 1
                    off += cwr

                agg = psA.tile([P, P], F32, space="PSUM", tag="agg")
                for c in range(cw):
                    oh = ohpool.tile([P, P], BF16, tag="oh")
                    nc.vector.scalar_tensor_tensor(
                        out=oh[:],
                        in0=iota_sb[:],
                        scalar=do_sb[:, c : c + 1],
                        in1=nq_sb[:, c : c + 1].to_broadcast([P, P]),
                        op0=mybir.AluOpType.is_equal,
                        op1=mybir.AluOpType.mult,
                    )
                    nc.tensor.matmul(
                        out=agg[:],
                        lhsT=g_sb[:, c * P : (c + 1) * P],
                        rhs=oh[:],
                        start=(c == 0),
                        stop=(c == cw - 1),
                    )

                agg_sb = eppool.tile([P, P], F32, tag="agg_sb")
                nc.vector.tensor_copy(agg_sb[:], agg[:])
                h_ps = psB.tile([P, P], F32, space="PSUM", tag="h")
                nc.tensor.matmul(
                    out=h_ps[:], lhsT=wt_sb[:], rhs=agg_sb[:], start=True, stop=True
                )
                relu_sb = eppool.tile([P, P], F32, tag="relu")
                nc.scalar.activation(
                    out=relu_sb[:],
                    in_=h_ps[:],
                    func=mybir.ActivationFunctionType.Relu,
                    bias=bias_sb[:, 0:1],
                )
                o_ps = psC.tile([P, 1], F32, space="PSUM", tag="o")
                nc.tensor.matmul(
                    out=o_ps[:], lhsT=relu_sb[:], rhs=linw_sb[:], start=True, stop=True
                )
                o_sb = eppool.tile([P, 1], F32, tag="osb")
                nc.scalar.activation(
                    out=o_sb[:],
                    in_=o_ps[:],
                    func=mybir.ActivationFunctionType.Sigmoid,
                    bias=linb_sb[:, 0:1],
                )
                nc.sync.dma_start(out=out[w * P : (w + 1) * P, :], in_=o_sb[:])
                cbase += cw

    nc.compile()
    return nc


_CACHE = {}


def _get_compiled(x, edge_index, W, conv_bias, lin_w, lin_b):
    cpwr, in_maps, npc, nwin = preprocess(x, edge_index, W, conv_bias, lin_w, lin_b)
    key = (x.shape, edge_index.shape, cpwr.tobytes())
    if key not in _CACHE:
        _CACHE[key] = build(cpwr, x.shape[0])
    return _CACHE[key], npc, in_maps


def kernel(x, edge_index, W, conv_bias, lin_w, lin_b):
    x = np.asarray(x)
    edge_index = np.asarray(edge_index)
    nc, npc, in_maps = _get_compiled(x, edge_index, W, conv_bias, lin_w, lin_b)
    res = run_bass_kernel_spmd(nc, in_maps, core_ids=list(range(NCORES)))
    N = x.shape[0]
    parts = [res.results[c]["out"][: min(npc, N - c * npc)] for c in range(NCORES)]
    return np.concatenate(parts, axis=0).astype(np.float32)

